# revision 22
# baseline (speedup 1.0000x reference)
"""ClusterMemoryCenter loss on 8 Trainium2 NeuronCores.

loss = 0.5 * (ce(hard_logits) + relu(ce(mean_logits) - R)) where
logits = inputs @ features.T / TEMP, features split into centroid/hard halves.

Sharding: the 2N=131072 feature rows are split along the cluster axis; each
of the 8 cores streams its 16384-row slice (8192 centroid + 8192 hard,
pre-transposed to [D, rows] bf16 on host) and computes per-B-row partial
sums of exp(logit) per column chunk (ACT exp with fused row-accumulate out
of 4-bank PSUM chunks). Host combines partials into the two log-sum-exps;
the 256 target logits are dot products done on host in f64.

bf16 operands: the CE averages over 256 rows and ~10k-effective softmax
terms, washing quantization noise out to ~3e-6 relative on the loss
(measured against the f32 reference).
"""

import numpy as np

TEMP = 0.05
R = 0.2
B, D, N = 256, 256, 65536
NCORES = 8
HALF_PER_CORE = N // NCORES            # 8192 rows of each half per core
ROWS_PER_CORE = 2 * HALF_PER_CORE      # 16384
SUB = 512                              # matmul N / one PSUM bank (f32)
# DMA blocks (feature rows each): small head blocks so the PE starts early,
# 4096-row (1 MiB per K-half) steady-state blocks for DMA efficiency, small
# tail blocks so the last exp chunk is short.
BLOCKS = [512, 1536, 2048, 2048, 2048, 2048, 2048, 2048, 1536, 512]
CHUNK = 2048                           # ACT chunk: 4 PSUM banks
# per-m accumulator columns, one per (block, chunk) pair
CHUNKS = []
for _w in BLOCKS:
    CHUNKS += [min(_w, CHUNK)] * max(1, _w // CHUNK)
NCOLS = len(CHUNKS)                    # 10
MEAN_COLS = 5                          # CHUNKS[:5] cover rows [0, 8192)
assert sum(CHUNKS) == ROWS_PER_CORE and sum(CHUNKS[:MEAN_COLS]) == HALF_PER_CORE

_cache = {}


def _build_nc():
    import concourse.bacc as bacc
    import concourse.mybir as mybir
    import concourse.tile as tile

    f32 = mybir.dt.float32
    bf16 = mybir.dt.bfloat16

    nc = bacc.Bacc("TRN2", target_bir_lowering=False, debug=False,
                   num_devices=NCORES)
    xT = nc.dram_tensor("xT", [D, B], bf16, kind="ExternalInput").ap()
    # fT is laid out block-major on host: for each DMA block, the [128, w]
    # K-half sub-matrix is a single contiguous DRAM range, so every feature
    # DMA is one fully linear HBM read (strided reads measured ~194 GB/s
    # aggregate; linear reads approach the ~358 GB/s per-core HBM limit).
    fT = nc.dram_tensor("fT", [D * ROWS_PER_CORE], bf16,
                        kind="ExternalInput").ap()
    osum = nc.dram_tensor("osum", [B, NCOLS], f32, kind="ExternalOutput").ap()

    with tile.TileContext(nc) as tc:
        with (
            tc.tile_pool(name="xpool", bufs=1) as xpool,
            tc.tile_pool(name="fpool", bufs=4) as fpool,
            tc.tile_pool(name="epool", bufs=6) as epool,
            tc.tile_pool(name="spool", bufs=1) as spool,
            tc.tile_pool(name="ppool", bufs=2, space="PSUM") as ppool,
        ):
            # inputs^T as two K-tiles [128, B]; columns m*128:(m+1)*128 are
            # the stationary operand for output-row block m
            # xT via the scalar engine's HWDGE queue so the sync queue's
            # first issues are the feature blocks the PE is waiting on
            xt = []
            for k in range(2):
                t = xpool.tile([128, B], bf16, name=f"xt{k}")
                nc.scalar.dma_start(t[:], xT[k * 128:(k + 1) * 128, :])
                xt.append(t)

            stats = [spool.tile([128, NCOLS], f32, name=f"stats{m}")
                     for m in range(2)]

            col = 0
            base = 0
            off = 0
            for blk_w in BLOCKS:
                fk = []
                for k in range(2):
                    t = fpool.tile([128, blk_w], bf16, name=f"f{k}",
                                   tag=f"f{k}")
                    src = fT[off:off + 128 * blk_w].rearrange(
                        "(p w) -> p w", p=128)
                    nc.sync.dma_start(t[:], src)
                    off += 128 * blk_w
                    fk.append(t)
                for coff in range(0, blk_w, CHUNK):
                    cw = min(CHUNK, blk_w - coff)
                    for m in range(2):
                        pt = ppool.tile([128, CHUNK], f32, name="pt",
                                        tag="pt")
                        for s in range(0, cw, SUB):
                            for k in range(2):
                                nc.tensor.matmul(
                                    pt[:, s:s + SUB],
                                    xt[k][:, m * 128:(m + 1) * 128],
                                    fk[k][:, coff + s:coff + s + SUB],
                                    start=(k == 0), stop=(k == 1))
                        et = epool.tile([128, CHUNK], bf16, name="et",
                                        tag="et")
                        # Balance the row-sum between the two engines: the
                        # otherwise-idle DVE reduces 3 of every 4 steady
                        # 2048-chunks (1x uop, ~2.3us/chunk); ACT keeps its
                        # fused accumulate for the rest so DVE never becomes
                        # the pipeline pacer.
                        if cw == CHUNK and col % 4 != 3:
                            nc.scalar.activation(
                                et[:, :cw], pt[:, :cw],
                                mybir.ActivationFunctionType.Exp,
                                scale=1.0 / TEMP)
                            nc.vector.reduce_sum(
                                stats[m][:, col:col + 1], et[:, :cw],
                                axis=mybir.AxisListType.X)
                        else:
                            nc.scalar.activation(
                                et[:, :cw], pt[:, :cw],
                                mybir.ActivationFunctionType.Exp,
                                scale=1.0 / TEMP,
                                accum_out=stats[m][:, col:col + 1])
                    col += 1
                base += blk_w
            assert col == NCOLS and base == ROWS_PER_CORE

            for m in range(2):
                nc.sync.dma_start(osum[m * 128:(m + 1) * 128, :], stats[m][:])

    nc.compile()
    return nc


def _get_nc():
    if "nc" not in _cache:
        _cache["nc"] = _build_nc()
    return _cache["nc"]


def _to_bf16(a):
    import ml_dtypes
    return np.ascontiguousarray(a, dtype=np.float32).astype(ml_dtypes.bfloat16)


def _shard_features(features):
    """Per-core flat bf16 arrays: the core's 8192 centroid + 8192 hard rows,
    transposed to [D, rows], then re-packed block-major so each (block,
    K-half) [128, w] tile is one contiguous DRAM range."""
    shards = []
    for c in range(NCORES):
        lo, hi = c * HALF_PER_CORE, (c + 1) * HALF_PER_CORE
        sl = np.concatenate([features[lo:hi], features[N + lo:N + hi]], axis=0)
        fTc = _to_bf16(np.ascontiguousarray(sl.T))   # [D, ROWS_PER_CORE]
        parts = []
        base = 0
        for w in BLOCKS:
            blkT = fTc[:, base:base + w]             # [256, w]
            parts.append(blkT[0:128].ravel())
            parts.append(blkT[128:256].ravel())
            base += w
        shards.append(np.ascontiguousarray(np.concatenate(parts)))
    return shards


def kernel(inputs, targets, features):
    from concourse.bass_utils import run_bass_kernel_spmd

    inputs = np.ascontiguousarray(np.asarray(inputs, dtype=np.float32))
    features = np.ascontiguousarray(np.asarray(features, dtype=np.float32))
    tgt = np.asarray(targets).astype(np.int64)

    xT = _to_bf16(inputs.T)
    in_maps = [{"xT": xT, "fT": fTk} for fTk in _shard_features(features)]

    nc = _get_nc()
    res = run_bass_kernel_spmd(nc, in_maps, core_ids=list(range(NCORES)))

    # combine partial sums: osum[b, c] = sum_j exp(logits[b, j]) over chunk c
    s_mean = np.zeros(B, np.float64)
    s_hard = np.zeros(B, np.float64)
    for r in res.results:
        osum = r["osum"].astype(np.float64)
        s_mean += osum[:, :MEAN_COLS].sum(axis=1)
        s_hard += osum[:, MEAN_COLS:].sum(axis=1)

    lse_mean = np.log(s_mean)
    lse_hard = np.log(s_hard)

    x64 = inputs.astype(np.float64)
    logit_t_mean = (x64 * features[tgt].astype(np.float64)).sum(axis=1) / TEMP
    logit_t_hard = (x64 * features[N + tgt].astype(np.float64)).sum(axis=1) / TEMP

    ce_mean = float(np.mean(lse_mean - logit_t_mean))
    ce_hard = float(np.mean(lse_hard - logit_t_hard))
    loss = 0.5 * (ce_hard + max(ce_mean - R, 0.0))
    return np.float32(loss)


# revision 28
# speedup vs baseline: 1.0523x; 1.0523x over previous
"""ClusterMemoryCenter loss on 8 Trainium2 NeuronCores.

loss = 0.5 * (ce(hard_logits) + relu(ce(mean_logits) - R)) where
logits = inputs @ features.T / TEMP, features split into centroid/hard halves.

Sharding: the 2N=131072 feature rows are split along the cluster axis; each
of the 8 cores streams its 16384-row slice (8192 centroid + 8192 hard,
pre-transposed to [D, rows] bf16 on host) and computes per-B-row partial
sums of exp(logit) per column chunk (ACT exp with fused row-accumulate out
of 4-bank PSUM chunks). Host combines partials into the two log-sum-exps;
the 256 target logits are dot products done on host in f64.

bf16 operands: the CE averages over 256 rows and ~10k-effective softmax
terms, washing quantization noise out to ~3e-6 relative on the loss
(measured against the f32 reference).
"""

import numpy as np

TEMP = 0.05
R = 0.2
B, D, N = 256, 256, 65536
NCORES = 8
HALF_PER_CORE = N // NCORES            # 8192 rows of each half per core
ROWS_PER_CORE = 2 * HALF_PER_CORE      # 16384
SUB = 512                              # matmul N / one PSUM bank (f32)
# DMA blocks (feature rows each): small head blocks so the PE starts early,
# 4096-row (1 MiB per K-half) steady-state blocks for DMA efficiency, small
# tail blocks so the last exp chunk is short.
BLOCKS = [512, 1536, 2048, 2048, 2048, 2048, 2048, 2048, 1536, 512]
CHUNK = 2048                           # ACT chunk: 4 PSUM banks
# per-m accumulator columns, one per (block, chunk) pair
CHUNKS = []
for _w in BLOCKS:
    CHUNKS += [min(_w, CHUNK)] * max(1, _w // CHUNK)
NCOLS = len(CHUNKS)                    # 10
MEAN_COLS = 5                          # CHUNKS[:5] cover rows [0, 8192)
assert sum(CHUNKS) == ROWS_PER_CORE and sum(CHUNKS[:MEAN_COLS]) == HALF_PER_CORE

_cache = {}


def _build_nc():
    import concourse.bacc as bacc
    import concourse.mybir as mybir
    import concourse.tile as tile

    f32 = mybir.dt.float32
    bf16 = mybir.dt.bfloat16

    nc = bacc.Bacc("TRN2", target_bir_lowering=False, debug=False,
                   num_devices=NCORES)
    # host-packed [128, 2B]: cols [0:B) = K-rows 0-127, [B:2B) = K-rows
    # 128-255, so one linear DMA loads the whole stationary operand
    xT = nc.dram_tensor("xT", [128, 2 * B], bf16, kind="ExternalInput").ap()
    # fT is laid out block-major on host: for each DMA block, the [128, w]
    # K-half sub-matrix is a single contiguous DRAM range, so every feature
    # DMA is one fully linear HBM read (strided reads measured ~194 GB/s
    # aggregate; linear reads approach the ~358 GB/s per-core HBM limit).
    fT = nc.dram_tensor("fT", [D * ROWS_PER_CORE], bf16,
                        kind="ExternalInput").ap()
    osum = nc.dram_tensor("osum", [B, NCOLS], f32, kind="ExternalOutput").ap()

    with tile.TileContext(nc) as tc:
        with (
            tc.tile_pool(name="xpool", bufs=1) as xpool,
            tc.tile_pool(name="fpool", bufs=4) as fpool,
            tc.tile_pool(name="epool", bufs=6) as epool,
            tc.tile_pool(name="spool", bufs=1) as spool,
            tc.tile_pool(name="ppool", bufs=2, space="PSUM") as ppool,
        ):
            # inputs^T as two K-tiles [128, B]; columns m*128:(m+1)*128 are
            # the stationary operand for output-row block m
            # xT via the scalar engine's HWDGE queue so the sync queue's
            # first issues are the feature blocks the PE is waiting on.
            # xt[k] view: columns k*B + m*128 hold the K-half-k stationary
            # operand for output-row block m.
            xtile = xpool.tile([128, 2 * B], bf16, name="xtile")
            nc.scalar.dma_start(xtile[:], xT[:, :])
            xt = [xtile[:, 0:B], xtile[:, B:2 * B]]

            stats = [spool.tile([128, NCOLS], f32, name=f"stats{m}")
                     for m in range(2)]

            # PE warm-up: ~8 dummy matmuls on zeroed SBUF while the first
            # feature DMAs are in flight, so HAM un-throttles (1.2->2.4 GHz)
            # before the first real matmul. Results are discarded.
            wtile = xpool.tile([128, SUB], bf16, name="wtile")
            nc.gpsimd.memset(wtile[:], 0.0)
            for _ in range(8):
                wp = ppool.tile([128, CHUNK], f32, name="wp", tag="pt")
                nc.tensor.matmul(wp[:, :SUB], wtile[:, :128], wtile[:],
                                 start=True, stop=True)

            col = 0
            base = 0
            off = 0
            for blk_w in BLOCKS:
                # one linear DMA per block: cols [0:w) = K-half 0, [w:2w) =
                # K-half 1 (host packs them adjacently)
                ft = fpool.tile([128, 2 * blk_w], bf16, name="ft", tag="ft")
                src = fT[off:off + 256 * blk_w].rearrange(
                    "(p w) -> p w", p=128)
                nc.sync.dma_start(ft[:], src)
                off += 256 * blk_w
                fk = [ft[:, 0:blk_w], ft[:, blk_w:2 * blk_w]]
                for coff in range(0, blk_w, CHUNK):
                    cw = min(CHUNK, blk_w - coff)
                    for m in range(2):
                        pt = ppool.tile([128, CHUNK], f32, name="pt",
                                        tag="pt")
                        for s in range(0, cw, SUB):
                            for k in range(2):
                                nc.tensor.matmul(
                                    pt[:, s:s + SUB],
                                    xt[k][:, m * 128:(m + 1) * 128],
                                    fk[k][:, coff + s:coff + s + SUB],
                                    start=(k == 0), stop=(k == 1))
                        et = epool.tile([128, CHUNK], bf16, name="et",
                                        tag="et")
                        # Balance the row-sum between the two engines: the
                        # otherwise-idle DVE reduces 3 of every 4 steady
                        # 2048-chunks (1x uop, ~2.3us/chunk); ACT keeps its
                        # fused accumulate for the rest so DVE never becomes
                        # the pipeline pacer.
                        if cw == CHUNK and col % 4 != 3:
                            nc.scalar.activation(
                                et[:, :cw], pt[:, :cw],
                                mybir.ActivationFunctionType.Exp,
                                scale=1.0 / TEMP)
                            nc.vector.reduce_sum(
                                stats[m][:, col:col + 1], et[:, :cw],
                                axis=mybir.AxisListType.X)
                        else:
                            nc.scalar.activation(
                                et[:, :cw], pt[:, :cw],
                                mybir.ActivationFunctionType.Exp,
                                scale=1.0 / TEMP,
                                accum_out=stats[m][:, col:col + 1])
                    col += 1
                base += blk_w
            assert col == NCOLS and base == ROWS_PER_CORE

            for m in range(2):
                nc.sync.dma_start(osum[m * 128:(m + 1) * 128, :], stats[m][:])

    nc.compile()
    return nc


def _get_nc():
    if "nc" not in _cache:
        _cache["nc"] = _build_nc()
    return _cache["nc"]


def _to_bf16(a):
    import ml_dtypes
    return np.ascontiguousarray(a, dtype=np.float32).astype(ml_dtypes.bfloat16)


def _shard_features(features):
    """Per-core flat bf16 arrays: the core's 8192 centroid + 8192 hard rows,
    transposed to [D, rows], then re-packed block-major so each (block,
    K-half) [128, w] tile is one contiguous DRAM range."""
    shards = []
    for c in range(NCORES):
        lo, hi = c * HALF_PER_CORE, (c + 1) * HALF_PER_CORE
        sl = np.concatenate([features[lo:hi], features[N + lo:N + hi]], axis=0)
        fTc = _to_bf16(np.ascontiguousarray(sl.T))   # [D, ROWS_PER_CORE]
        parts = []
        base = 0
        for w in BLOCKS:
            blkT = fTc[:, base:base + w]             # [256, w]
            # [128, 2w]: partition p = [K-half-0 row p, K-half-1 row p]
            parts.append(np.concatenate([blkT[0:128], blkT[128:256]],
                                        axis=1).ravel())
            base += w
        shards.append(np.ascontiguousarray(np.concatenate(parts)))
    return shards


def _prep_in_maps(inputs, features):
    xTd = _to_bf16(inputs.T)                          # [D, B]
    xT = np.ascontiguousarray(
        np.concatenate([xTd[0:128], xTd[128:256]], axis=1))  # [128, 2B]
    return [{"xT": xT, "fT": fTk} for fTk in _shard_features(features)]


def kernel(inputs, targets, features):
    from concourse.bass_utils import run_bass_kernel_spmd

    inputs = np.ascontiguousarray(np.asarray(inputs, dtype=np.float32))
    features = np.ascontiguousarray(np.asarray(features, dtype=np.float32))
    tgt = np.asarray(targets).astype(np.int64)

    in_maps = _prep_in_maps(inputs, features)

    nc = _get_nc()
    res = run_bass_kernel_spmd(nc, in_maps, core_ids=list(range(NCORES)))

    # combine partial sums: osum[b, c] = sum_j exp(logits[b, j]) over chunk c
    s_mean = np.zeros(B, np.float64)
    s_hard = np.zeros(B, np.float64)
    for r in res.results:
        osum = r["osum"].astype(np.float64)
        s_mean += osum[:, :MEAN_COLS].sum(axis=1)
        s_hard += osum[:, MEAN_COLS:].sum(axis=1)

    lse_mean = np.log(s_mean)
    lse_hard = np.log(s_hard)

    x64 = inputs.astype(np.float64)
    logit_t_mean = (x64 * features[tgt].astype(np.float64)).sum(axis=1) / TEMP
    logit_t_hard = (x64 * features[N + tgt].astype(np.float64)).sum(axis=1) / TEMP

    ce_mean = float(np.mean(lse_mean - logit_t_mean))
    ce_hard = float(np.mean(lse_hard - logit_t_hard))
    loss = 0.5 * (ce_hard + max(ce_mean - R, 0.0))
    return np.float32(loss)
